# revision 1
# baseline (speedup 1.0000x reference)
"""Trainium2 Bass kernel for nn_Attention_9122510537215 (gnn_message_passing).

Math (per batch b):
    Q = query @ Wq.T + bq                  [LQ=256, 256]
    K = input @ Wk.T + bk                  [LK, 256]
    V = input @ Wv.T + bv                  [LK, 256]
    alpha = softmax_k(Q @ K.T / 16)        [256, LK]
    out[j] = sum_k alpha[j, k] * V[k, j]   [256]

Algebraic restructuring used here:
  * bk shifts every score column by a constant along k -> cancels in softmax_k.
  * G[b] = Wk.T @ (query_b @ Wq.T + bq).T / 16, so scoresT = input @ G  ([LK, 256]).
  * Instead of materializing V, accumulate H[j, i] = sum_k e[k, j] * input[k, i]
    (e = exp(scores)); then numer[j] = sum_i H[j, i] * Wv[j, i] and an appended
    ones-column of the input yields denom[j] = H[j, 256].  bv is applied at the
    end:  out = numer / denom + bv.
  * Softmax is computed unnormalized without max-subtraction (scores are O(1)
    for this problem family; exp stays within a safe range).

Distribution: the LK (node) axis is zero-padded to 50176 = 8 * 6272 and sharded
across the 8 NeuronCores; each core returns its partial H accumulators
([B, 2, 128, 257] fp32) and the host reduces across cores in float64.
Padded rows carry a 0 in the ones-column so they contribute nothing.

Device layout: the host pre-casts the input to fp16 and ships both layouts so
the device does no transposes or casts:
  * "xn": natural rows, tile-transposed as [B, 128(part), 49(subtile), 258]
    so each partition reads one long contiguous run per DMA (>=4KB
    descriptors — descriptor rate, not bytes, limits the DMA engines).
    Node identity: k = subtile*128 + partition.
  * "xt": transposed [B, 256(i), 6272(k)] — k-contiguous per feature row.
TensorE streams fp16 matmuls (scores + H), ScalarE does exp, VectorE idle.
"""

import numpy as np
from contextlib import ExitStack

import concourse.bass as bass
import concourse.mybir as mybir
import concourse.tile as tile
from concourse import bacc
from concourse.bass_utils import run_bass_kernel_spmd

# Problem constants (hardcoded; kernel.py must be self-contained).
B = 4
LQ = 256
LK = 50000
OUT = 256
KV = 256            # input feature dim
NORM = 1.0 / 16.0   # 1/sqrt(OUT)

N_CORES = 8
SUB = 128                  # nodes per subtile (PE contraction width)
NSUB = 49                  # subtiles per core per batch
KS = NSUB * SUB            # 6272 nodes per core per batch
LK_PAD = KS * N_CORES      # 50176
CGRP = 4                   # subtiles per compute group (exp batching / PSUM)
DGRP = 8                   # subtiles per DMA group (descriptor batching)

F16 = mybir.dt.float16
F32 = mybir.dt.float32


def _round_robin(nc, names):
    state = [0]
    def pick():
        e = getattr(nc, names[state[0] % len(names)])
        state[0] += 1
        return e
    return pick


def build(ks=KS, repeat=1, dma_engines=("gpsimd", "sync", "scalar"),
          nat_bufs=3, tp_bufs=3, e_bufs=3, mode="full", dgrp=DGRP):
    """Emit the per-core SPMD Bass module (identical on all cores).

    repeat > 1 wraps the body in a hardware For_i loop recomputing the same
    result — used only for wall-clock benchmarking.
    mode: "full" (normal), "dma" (input loads only), "compute" (static inputs,
    no streaming loads) — ablation benchmarks.
    """
    nsub = ks // SUB
    cgroups = [CGRP] * (nsub // CGRP)
    if nsub % CGRP:
        cgroups.append(nsub % CGRP)

    nc = bacc.Bacc("TRN2", target_bir_lowering=False, debug=False,
                   num_devices=N_CORES)
    xn = nc.dram_tensor("xn", [B, 128, nsub, 258], F16, kind="ExternalInput")
    xt = nc.dram_tensor("xt", [B, 256, ks], F16, kind="ExternalInput")
    g = nc.dram_tensor("g", [B, 256, 256], F16, kind="ExternalInput")
    ht = nc.dram_tensor("ht", [B, 2, 128, 257], F32, kind="ExternalOutput")

    with ExitStack() as ctx:
        tc = ctx.enter_context(tile.TileContext(nc))
        gp = ctx.enter_context(tc.tile_pool(name="gp", bufs=1))
        natp = ctx.enter_context(tc.tile_pool(name="natp", bufs=nat_bufs))
        tpp = ctx.enter_context(tc.tile_pool(name="tpp", bufs=tp_bufs))
        ep = ctx.enter_context(tc.tile_pool(name="ep", bufs=e_bufs))
        hout = ctx.enter_context(tc.tile_pool(name="hout", bufs=2))
        spp = ctx.enter_context(tc.tile_pool(name="spp", bufs=2, space="PSUM"))
        hpp = ctx.enter_context(tc.tile_pool(name="hpp", bufs=2, space="PSUM"))

        # G for all batches, resident in SBUF: [i(2x128 part), q(256)].
        g_sb = gp.tile([128, B, 2, 256], F16)
        for b in range(B):
            for ih in range(2):
                nc.sync.dma_start(out=g_sb[:, b, ih, :],
                                  in_=g[b, ih * 128:(ih + 1) * 128, :])

        static_tiles = None
        e_static = None
        if mode in ("compute", "mmonly"):
            stp = ctx.enter_context(tc.tile_pool(name="static", bufs=1))
            snat = stp.tile([128, dgrp, 258], F16, tag="snat")
            nc.sync.dma_start(out=snat[:, :, :], in_=xn[0, :, 0:dgrp, :])
            stps = []
            for ih in range(2):
                t_ = stp.tile([128, dgrp * SUB], F16, tag=f"stp{ih}")
                nc.sync.dma_start(out=t_[:, :],
                                  in_=xt[0, ih * 128:(ih + 1) * 128, 0:dgrp * SUB])
                stps.append(t_)
            static_tiles = (snat, stps[0], stps[1])
            if mode == "mmonly":
                e_static = stp.tile([128, CGRP, 256], F16, tag="se")
                nc.vector.memset(e_static[:, :, :], 1.0)

        if mode.startswith("mm") and mode != "mmonly":
            # Pure PE microbenchmark: same MM count as the real kernel (784),
            # parameterized moving width N, all-static operands.
            # mm256/mm512/mm128: one stationary reused.
            # mmfresh: rotate 8 stationaries as strided slices of one buffer.
            # mmfresh2: rotate 8 dense stationary tiles.
            N = {"mm512": 512, "mm128": 128}.get(mode, 256)
            stp2 = ctx.enter_context(tc.tile_pool(name="static2", bufs=1))
            if mode in ("mmfresh", "mmpair", "mmht"):
                wbuf = stp2.tile([128, 8, 128], F16, tag="w")
                nc.vector.memset(wbuf[:, :, :], 0.5)
                ws = [wbuf[:, j, :] for j in range(8)]
            elif mode == "mmfresh2":
                ws = []
                for j in range(8):
                    wtile = stp2.tile([128, 128], F16, tag=f"w{j}")
                    nc.vector.memset(wtile[:, :], 0.5)
                    ws.append(wtile[:, :])
            else:
                w_static = stp2.tile([128, 128], F16, tag="w")
                nc.vector.memset(w_static[:, :], 0.5)
                ws = [w_static[:, :]] * 8
            r_static = stp2.tile([128, N], F16, tag="r")
            nc.vector.memset(r_static[:, :], 0.5)
            r258 = stp2.tile([128, 8, 258], F16, tag="r258")
            nc.vector.memset(r258[:, :, :], 0.5)
            spp2 = ctx.enter_context(
                tc.tile_pool(name="psum2", bufs=4, space="PSUM"))
            hpp2 = ctx.enter_context(
                tc.tile_pool(name="hpsum2", bufs=2, space="PSUM"))
            rep_ctx = tc.For_i(0, repeat, 1) if repeat > 1 else None
            if rep_ctx is not None:
                ctx.enter_context(rep_ctx)
            if mode == "mmpair":
                # scores-like: 2-MM accumulation pairs into rotating half-banks
                for grp in range(98):
                    o = spp2.tile([128, 4, 256], F32, tag="o4")
                    for i in range(4):
                        nc.tensor.matmul(o[:, i, :], ws[2 * i], r_static[:, :],
                                         start=True, stop=False)
                        nc.tensor.matmul(o[:, i, :], ws[2 * i + 1],
                                         r_static[:, :],
                                         start=False, stop=True)
            elif mode == "mmht":
                # HT-like: two persistent accumulators, N=257, alternating
                for rep_b in range(4):
                    a0 = hpp2.tile([128, 257], F32, tag="a0")
                    a1 = hpp2.tile([128, 257], F32, tag="a1")
                    for grp in range(49):
                        for i in range(2):
                            first = grp == 0 and i == 0
                            last = grp == 48 and i == 1
                            nc.tensor.matmul(a0[:, :], ws[2 * i],
                                             r258[:, 2 * i, 0:257],
                                             start=first, stop=last)
                            nc.tensor.matmul(a1[:, :], ws[2 * i + 1],
                                             r258[:, 2 * i + 1, 0:257],
                                             start=first, stop=last)
            else:
                for grp in range(98):
                    o = spp2.tile([128, N], F32, tag="o")
                    for j in range(8):
                        nc.tensor.matmul(o[:, :], ws[j], r_static[:, :],
                                         start=(j == 0), stop=(j == 7))
        else:
            rep_ctx = tc.For_i(0, repeat, 1) if repeat > 1 else None
            if rep_ctx is not None:
                ctx.enter_context(rep_ctx)

        n_cg = len(cgroups)
        pick_eng = _round_robin(nc, dma_engines)
        n_batches = 0 if (mode.startswith("mm") and mode != "mmonly") else B
        for b in range(n_batches):
            ht0 = hpp.tile([128, 257], F32, tag="ht0")
            ht1 = hpp.tile([128, 257], F32, tag="ht1")
            dma_tiles = {}  # dma-group index -> (nat, tp0, tp1)

            def load_dgroup(d):
                if mode == "compute":
                    dma_tiles[d] = static_tiles
                    return
                d0 = d * dgrp
                dsz = min(dgrp, nsub - d0)
                natt = natp.tile([128, dgrp, 258], F16, tag="nat")
                pick_eng().dma_start(out=natt[:, :dsz, :],
                                     in_=xn[b, :, d0:d0 + dsz, :])
                tps = []
                for ih in range(2):
                    tptt = tpp.tile([128, dgrp * SUB], F16, tag=f"tp{ih}")
                    pick_eng().dma_start(
                        out=tptt[:, :dsz * SUB],
                        in_=xt[b, ih * 128:(ih + 1) * 128,
                               d0 * SUB:(d0 + dsz) * SUB])
                    tps.append(tptt)
                dma_tiles[d] = (natt, tps[0], tps[1])

            pend = None  # (e, subtile_list, is_first) of previous compute group
            for t in range(n_cg + 1):
                if t < n_cg:
                    sz = cgroups[t]
                    subs = [t * CGRP + i for i in range(sz)]
                    for s in subs:
                        if s // dgrp not in dma_tiles:
                            load_dgroup(s // dgrp)
                    if mode == "dma":
                        continue
                    # scoresT[k, q] = sum_i inpT[i, k].T @ G[i, q]
                    sp = spp.tile([128, CGRP, 256], F32)
                    for i, s in enumerate(subs):
                        natt, tp0, tp1 = dma_tiles[s // dgrp]
                        r = s % dgrp
                        nc.tensor.matmul(sp[:, i, :],
                                         tp0[:, r * SUB:(r + 1) * SUB],
                                         g_sb[:, b, 0, :],
                                         start=True, stop=False)
                        nc.tensor.matmul(sp[:, i, :],
                                         tp1[:, r * SUB:(r + 1) * SUB],
                                         g_sb[:, b, 1, :],
                                         start=False, stop=True)
                    if mode == "mmonly":
                        e = e_static
                    else:
                        e = ep.tile([128, CGRP, 256], F16)
                        nc.scalar.activation(e[:, :sz, :], sp[:, :sz, :],
                                             mybir.ActivationFunctionType.Exp)
                # H matmuls of the previous compute group (keeps PE busy while
                # ScalarE computes this group's exp).
                if mode == "scores":
                    pend = None
                if pend is not None:
                    pe_, psubs, pfirst = pend
                    for i, s in enumerate(psubs):
                        natt = dma_tiles[s // dgrp][0]
                        is_first = pfirst and i == 0
                        is_last = (t == n_cg) and i == len(psubs) - 1
                        nc.tensor.matmul(ht0[:, :], pe_[:, i, 0:128],
                                         natt[:, s % dgrp, 0:257],
                                         start=is_first, stop=is_last)
                        nc.tensor.matmul(ht1[:, :], pe_[:, i, 128:256],
                                         natt[:, s % dgrp, 0:257],
                                         start=is_first, stop=is_last)
                if t < n_cg:
                    pend = (e, subs, t == 0)
            if mode in ("dma", "scores"):
                continue
            hts = hout.tile([128, 2, 257], F32)
            nc.vector.tensor_copy(hts[:, 0, :], ht0[:, :])
            nc.vector.tensor_copy(hts[:, 1, :], ht1[:, :])
            nc.sync.dma_start(out=ht[b, 0], in_=hts[:, 0, :])
            nc.sync.dma_start(out=ht[b, 1], in_=hts[:, 1, :])
    nc.compile()
    return nc


def _prepare_inputs(query, input, Wq, bq, Wk):
    """Host-side marshalling: G matrices + fp16 input in both layouts, sharded."""
    # G[b] = Wk.T @ (query_b @ Wq.T + bq).T * NORM   -> [B, 256(i), 256(q)]
    Q = query.astype(np.float64) @ Wq.T.astype(np.float64) + bq
    G = np.einsum('di,bqd->biq', Wk.astype(np.float64), Q) * NORM
    g16 = np.ascontiguousarray(G.astype(np.float32).astype(np.float16))

    xn = np.zeros((B, LK_PAD, 258), np.float16)
    xn[:, :LK, :256] = input.astype(np.float16)
    xn[:, :LK, 256] = 1.0   # ones-column -> denom; stays 0 on padded rows
    xt_view = xn[:, :, :256].transpose(0, 2, 1)  # [B, 256, LK_PAD] view

    in_maps = []
    for c in range(N_CORES):
        sl = slice(c * KS, (c + 1) * KS)
        # natural, tile-transposed: [B, 128, NSUB, 258]; node k = t*128 + p
        xn_c = xn[:, sl, :].reshape(B, NSUB, 128, 258).transpose(0, 2, 1, 3)
        in_maps.append({
            "xn": np.ascontiguousarray(xn_c),
            "xt": np.ascontiguousarray(xt_view[:, :, sl]),
            "g": g16,
        })
    return in_maps


def kernel(query, input, Wq, bq, Wk, bk, Wv, bv):
    # bk provably cancels in softmax over k; bq is folded into G; bv is applied
    # in the host-side epilogue below.
    query = np.asarray(query, dtype=np.float32)
    input = np.asarray(input, dtype=np.float32)
    Wq = np.asarray(Wq, dtype=np.float32)
    bq = np.asarray(bq, dtype=np.float32)
    Wk = np.asarray(Wk, dtype=np.float32)
    Wv = np.asarray(Wv, dtype=np.float32)
    bv = np.asarray(bv, dtype=np.float32)

    nc = build()
    in_maps = _prepare_inputs(query, input, Wq, bq, Wk)
    res = run_bass_kernel_spmd(nc, in_maps, core_ids=list(range(N_CORES)))
    kernel._last_result = res

    numer = np.zeros((B, OUT))
    denom = np.zeros((B, OUT))
    Wv64 = Wv.astype(np.float64)
    for r in res.results:
        H = r["ht"].astype(np.float64).reshape(B, OUT, 257)  # j = half*128 + p
        numer += (H[:, :, :256] * Wv64[None]).sum(axis=2)
        denom += H[:, :, 256]
    out = numer / denom + bv
    return out.astype(np.float32)


if __name__ == "__main__":
    # CoreSim smoke test on a reduced size (5 subtiles -> cgroups [4, 1]).
    from concourse.bass_interp import CoreSim

    nsub_t = 5
    ks = nsub_t * SUB
    rng = np.random.default_rng(0)
    xn_np = rng.standard_normal((B, ks, 258)).astype(np.float16)
    xn_np[:, :, 256] = 1.0
    xn_np[:, :, 257] = 0.0
    xt_np = np.ascontiguousarray(xn_np[:, :, :256].transpose(0, 2, 1))
    xn_tiled = np.ascontiguousarray(
        xn_np.reshape(B, nsub_t, 128, 258).transpose(0, 2, 1, 3))
    g_np = (rng.standard_normal((B, 256, 256)) * 0.05).astype(np.float16)

    nc = build(ks=ks)
    sim = CoreSim(nc)
    sim.tensor("xn")[:] = xn_tiled
    sim.tensor("xt")[:] = xt_np
    sim.tensor("g")[:] = g_np
    sim.simulate()
    got = np.array(sim.tensor("ht")).reshape(B, OUT, 257)

    x = xn_np[:, :, :257].astype(np.float32)
    want = np.zeros((B, OUT, 257), np.float32)
    for b in range(B):
        s = x[b, :, :256] @ g_np[b].astype(np.float32)
        e = np.exp(s).astype(np.float16).astype(np.float32)
        want[b] = e.T @ x[b]
    err = np.abs(got - want).max() / np.abs(want).max()
    print("CoreSim rel err:", err)
    assert err < 2e-2, err
    print("OK")



# revision 3
# speedup vs baseline: 1.7143x; 1.7143x over previous
"""Trainium2 Bass kernel for nn_Attention_9122510537215 (gnn_message_passing).

Math (per batch b):
    Q = query @ Wq.T + bq                  [LQ=256, 256]
    K = input @ Wk.T + bk                  [LK, 256]
    V = input @ Wv.T + bv                  [LK, 256]
    alpha = softmax_k(Q @ K.T / 16)        [256, LK]
    out[j] = sum_k alpha[j, k] * V[k, j]   [256]

Algebraic restructuring:
  * bk shifts every score column by a constant along k -> cancels in softmax_k.
  * G[b] = Wk.T @ (query_b @ Wq.T + bq).T / 16, so scoresT = input @ G  ([LK, 256]).
  * Instead of materializing V, accumulate H[j, i] = sum_k e[k, j] * input[k, i]
    (e = exp(scores)); numer[j] = sum_i H[j, i] * Wv[j, i]; an appended
    ones-column yields denom[j] = H[j, 256]; out = numer / denom + bv.
  * Softmax is computed unnormalized without max-subtraction (scores are O(1)).

Performance structure (vs the fp16 predecessor):
  * All matmuls run in fp8e4 (e4m3) with MatmulPerfMode.DoubleRow: two
    128-row contraction tiles per pass at 0.5 cycles/output-column.
      - scores: per 128-node subtile, ONE DR matmul contracts all 256 input
        features (xt laid out [i_lo(128 part), 2(i_hi), k]).
      - H: subtile PAIRS contract 256 nodes per pass (e laid out
        [k(128 part), 2(pair), j]; xn natural [k, 2(pair), 257]).
  * exp is the serial bottleneck (B*LQ*LK/8 = 6.4M exps/core), so it is
    SPLIT across two engines: ScalarE computes exact Exp (fp8 out,
    scale=1/SG), and VectorE computes a Schraudolph-style exp by writing
    round(A8*score + B8) as int8 and BITCASTING those bytes as fp8e4
    (weights' 3.3% rms wiggle averages out over 50k softmax terms).
  * Whole per-core input (12.9 MB fp8) is SBUF-resident; chunked DMAs on a
    single ordered sync queue overlap the whole compute pipeline.
  * Distribution: LK padded to 50176 = 8*6272, sharded over 8 cores; each
    core returns fp16 partial H [B, 2, 128, 257]; host reduces in float64.
"""

import numpy as np
from contextlib import ExitStack

import concourse.bass as bass
import concourse.mybir as mybir
import concourse.tile as tile
from concourse import bacc
from concourse.bass_utils import run_bass_kernel_spmd

# Problem constants (hardcoded; kernel.py must be self-contained).
B = 4
LQ = 256
LK = 50000
OUT = 256
KV = 256            # input feature dim
NORM = 1.0 / 16.0   # 1/sqrt(OUT)

N_CORES = 8
SUB = 128                  # nodes per subtile (PE contraction width)
NSUB = 49                  # subtiles per core per batch
KS = NSUB * SUB            # 6272 nodes per core per batch
LK_PAD = KS * N_CORES      # 50176
GRP = 4                    # subtiles per exp/psum group (2 DoubleRow pairs)
NGRP = NSUB // GRP         # 12 full groups; subtile 48 is the odd tail
NCHUNK = 4                 # DMA chunks per (batch, layout)
CH = 12                    # subtiles per chunk (last chunk has 13)

SG = 64.0                        # score scale inside PSUM (folded into g)
A8 = 8 * np.log2(np.e) / SG      # Schraudolph fp8e4 slope
B8 = 56.05                       # 8*7 bias, +0.5 trunc->round, -0.45 mean-cal

# 7 ScalarE groups / 5 VectorE groups (+ odd tail on VectorE) per batch.
ACT_GROUPS = frozenset((0, 2, 4, 6, 7, 9, 11))

F8 = mybir.dt.float8e4
F16 = mybir.dt.float16
F32 = mybir.dt.float32
I8 = mybir.dt.int8
NPF8 = mybir.dt.np(mybir.dt.float8e4)
DR = mybir.MatmulPerfMode.DoubleRow


def _chunk_of(s):
    """DMA chunk index and subtile offset within the chunk."""
    c = min(s // CH, NCHUNK - 1)
    return c, s - c * CH


def build():
    """Emit the per-core SPMD Bass module (identical on all cores)."""
    nc = bacc.Bacc("TRN2", target_bir_lowering=False, debug=False,
                   num_devices=N_CORES)
    xn = nc.dram_tensor("xn", [B, 128, NSUB, 258], F8, kind="ExternalInput")
    xt = nc.dram_tensor("xt", [B, 128, 2, KS], F8, kind="ExternalInput")
    g = nc.dram_tensor("g", [128, B, 2, 256], F8, kind="ExternalInput")
    ht = nc.dram_tensor("ht", [B, 2, 128, 257], F16, kind="ExternalOutput")

    with ExitStack() as ctx:
        tc = ctx.enter_context(tile.TileContext(nc))
        gp = ctx.enter_context(tc.tile_pool(name="gp", bufs=1))
        xnp = ctx.enter_context(tc.tile_pool(name="xnp", bufs=B * NCHUNK))
        xtp = ctx.enter_context(tc.tile_pool(name="xtp", bufs=B * NCHUNK))
        ep = ctx.enter_context(tc.tile_pool(name="ep", bufs=4))
        eip = ctx.enter_context(tc.tile_pool(name="eip", bufs=4))
        hop = ctx.enter_context(tc.tile_pool(name="hop", bufs=2))
        spp = ctx.enter_context(tc.tile_pool(name="spp", bufs=2, space="PSUM"))
        hpp = ctx.enter_context(tc.tile_pool(name="hpp", bufs=2, space="PSUM"))

        # G for all batches, resident: [i_lo(128 part), b, i_hi, q].
        g_sb = gp.tile([128, B, 2, 256], F8)
        nc.sync.dma_start(out=g_sb[:, :, :, :], in_=g[:, :, :, :])

        # Chunked input loads, one ordered queue (sync/SP), interleaved so
        # batch 0's first chunks land first.
        xn_tiles = {}
        xt_tiles = {}
        for b in range(B):
            for c in range(NCHUNK):
                s0 = c * CH
                ns = CH + 1 if c == NCHUNK - 1 else CH
                xtt = xtp.tile([128, 2, (CH + 1) * SUB], F8, tag="xt")
                nc.sync.dma_start(
                    out=xtt[:, :, :ns * SUB],
                    in_=xt[b, :, :, s0 * SUB:(s0 + ns) * SUB])
                xt_tiles[(b, c)] = xtt
                xnt = xnp.tile([128, CH + 1, 258], F8, tag="xn")
                nc.sync.dma_start(out=xnt[:, :ns, :],
                                  in_=xn[b, :, s0:s0 + ns, :])
                xn_tiles[(b, c)] = xnt

        # groups: (subtile list, is_act) per batch; odd tail rides VectorE.
        groups = [([g_ * GRP + i for i in range(GRP)], g_ in ACT_GROUPS)
                  for g_ in range(NGRP)]
        groups.append(([NSUB - 1], False))

        for b in range(B):
            ht0 = hpp.tile([128, 257], F32, tag="ht0")
            ht1 = hpp.tile([128, 257], F32, tag="ht1")
            pend = None  # (e_f8, subs, is_first) awaiting H matmuls

            for t, item in enumerate(groups + [None]):
                subs, is_act = item if item is not None else (None, False)
                if subs is not None:
                    sz = len(subs)
                    sp = spp.tile([128, GRP, 256], F32)
                    for i, s in enumerate(subs):
                        c, off = _chunk_of(s)
                        xtt = xt_tiles[(b, c)]
                        nc.tensor.matmul(
                            sp[:, i, :],
                            xtt[:, :, off * SUB:(off + 1) * SUB],
                            g_sb[:, b, :, :],
                            start=True, stop=True, perf_mode=DR)
                    if is_act:
                        e = ep.tile([128, GRP, 256], F8, tag="ea")
                        nc.scalar.activation(
                            e[:, :sz, :], sp[:, :sz, :],
                            mybir.ActivationFunctionType.Exp, scale=1.0 / SG)
                        e_f8 = e
                    else:
                        ei = eip.tile([128, GRP, 256], I8, tag="ei")
                        nc.vector.tensor_scalar(
                            ei[:, :sz, :], sp[:, :sz, :], A8, B8,
                            mybir.AluOpType.mult, mybir.AluOpType.add)
                        e_f8 = ei[:, :, :].bitcast(F8)
                # H matmuls of the previous group (keeps PE behind the
                # exp engines without blocking the next scores).
                if pend is not None:
                    pe_, psubs, pfirst = pend
                    npair = len(psubs) // 2
                    for i in range(npair):
                        s = psubs[2 * i]
                        c, off = _chunk_of(s)
                        xnt = xn_tiles[(b, c)]
                        first = pfirst and i == 0
                        last = (subs is None) and i == npair - 1
                        for h, hacc in ((0, ht0), (1, ht1)):
                            nc.tensor.matmul(
                                hacc[:, :],
                                pe_[:, 2 * i:2 * i + 2, h * 128:(h + 1) * 128],
                                xnt[:, off:off + 2, 0:257],
                                start=first, stop=last, perf_mode=DR)
                    if len(psubs) % 2:  # odd tail: plain fp8 matmul
                        s = psubs[-1]
                        c, off = _chunk_of(s)
                        xnt = xn_tiles[(b, c)]
                        i = len(psubs) - 1
                        first = pfirst and npair == 0
                        last = subs is None
                        for h, hacc in ((0, ht0), (1, ht1)):
                            nc.tensor.matmul(
                                hacc[:, :],
                                pe_[:, i, h * 128:(h + 1) * 128],
                                xnt[:, off, 0:257],
                                start=first, stop=last)
                if subs is not None:
                    pend = (e_f8, subs, t == 0)

            hts = hop.tile([128, 2, 257], F16)
            nc.vector.tensor_copy(hts[:, 0, :], ht0[:, :])
            nc.vector.tensor_copy(hts[:, 1, :], ht1[:, :])
            nc.gpsimd.dma_start(out=ht[b, 0], in_=hts[:, 0, :])
            nc.gpsimd.dma_start(out=ht[b, 1], in_=hts[:, 1, :])
    nc.compile()
    return nc


def _prepare_inputs(query, input, Wq, bq, Wk):
    """Host marshalling: folded G + fp8 input in both layouts, k-sharded."""
    # G[b] = Wk.T @ (query_b @ Wq.T + bq).T -> [B, 256(i), 256(q)], then
    # * NORM (1/16) * SG (64) so PSUM scores arrive pre-scaled by SG.
    Q = query.astype(np.float64) @ Wq.T.astype(np.float64) + bq
    G = np.einsum('di,bqd->biq', Wk.astype(np.float64), Q) * (NORM * SG)
    # [i_lo, b, i_hi, q] with i = i_hi*128 + i_lo
    g8 = np.ascontiguousarray(
        G.astype(np.float32).reshape(B, 2, 128, 256).transpose(2, 0, 1, 3)
    ).astype(NPF8)

    xpad = np.zeros((B, LK_PAD, 258), np.float32)
    xpad[:, :LK, :256] = input
    xpad[:, :LK, 256] = 1.0   # ones-column -> denom; 0 on padded rows
    x8 = xpad.astype(NPF8)    # [B, LK_PAD, 258]

    in_maps = []
    for cid in range(N_CORES):
        sl = x8[:, cid * KS:(cid + 1) * KS, :]
        # natural: [B, 128(part), NSUB, 258]; node k = t*128 + p
        xn_c = sl.reshape(B, NSUB, 128, 258).transpose(0, 2, 1, 3)
        # transposed DoubleRow: [B, 128(i_lo), 2(i_hi), KS]
        xt_c = np.ascontiguousarray(
            sl[:, :, :256].transpose(0, 2, 1)).reshape(B, 2, 128, KS)
        xt_c = xt_c.transpose(0, 2, 1, 3)
        in_maps.append({
            "xn": np.ascontiguousarray(xn_c),
            "xt": np.ascontiguousarray(xt_c),
            "g": g8,
        })
    return in_maps


def kernel(query, input, Wq, bq, Wk, bk, Wv, bv):
    # bk provably cancels in softmax over k; bq folds into G; bv is applied
    # in the host epilogue.
    query = np.asarray(query, dtype=np.float32)
    input = np.asarray(input, dtype=np.float32)
    Wq = np.asarray(Wq, dtype=np.float32)
    bq = np.asarray(bq, dtype=np.float32)
    Wk = np.asarray(Wk, dtype=np.float32)
    Wv = np.asarray(Wv, dtype=np.float32)
    bv = np.asarray(bv, dtype=np.float32)

    nc = build()
    in_maps = _prepare_inputs(query, input, Wq, bq, Wk)
    res = run_bass_kernel_spmd(nc, in_maps, core_ids=list(range(N_CORES)))
    kernel._last_result = res

    numer = np.zeros((B, OUT))
    denom = np.zeros((B, OUT))
    Wv64 = Wv.astype(np.float64)
    for r in res.results:
        H = r["ht"].astype(np.float64).reshape(B, OUT, 257)  # j = h*128 + p
        numer += (H[:, :, :256] * Wv64[None]).sum(axis=2)
        denom += H[:, :, 256]
    out = numer / denom + bv
    return out.astype(np.float32)


if __name__ == "__main__":
    # CoreSim smoke test on the full module with random fp8 inputs.
    from concourse.bass_interp import CoreSim

    rng = np.random.default_rng(0)
    xpad = np.zeros((B, KS, 258), np.float32)
    xpad[:, :, :256] = rng.standard_normal((B, KS, 256))
    xpad[:, :, 256] = 1.0
    x8 = xpad.astype(NPF8)
    xn_np = np.ascontiguousarray(
        x8.reshape(B, NSUB, 128, 258).transpose(0, 2, 1, 3))
    xt_np = np.ascontiguousarray(
        x8[:, :, :256].transpose(0, 2, 1)).reshape(B, 2, 128, KS)
    xt_np = np.ascontiguousarray(xt_np.transpose(0, 2, 1, 3))
    g_np = (rng.standard_normal((B, 256, 256)) * 1.8).astype(np.float32)
    g8 = np.ascontiguousarray(
        g_np.reshape(B, 2, 128, 256).transpose(2, 0, 1, 3)).astype(NPF8)

    nc = build()
    sim = CoreSim(nc)
    sim.tensor("xn")[:] = xn_np
    sim.tensor("xt")[:] = xt_np
    sim.tensor("g")[:] = g8
    sim.simulate()
    got = np.array(sim.tensor("ht")).astype(np.float64).reshape(B, OUT, 257)

    x32 = x8.astype(np.float32)
    g32 = g8.astype(np.float32)  # [il, b, ih, q]
    want = np.zeros((B, OUT, 257))
    weird = 0.0
    for b in range(B):
        gb = g32[:, b, :, :].transpose(1, 0, 2).reshape(256, 256)
        s = x32[b, :, :256] @ gb  # = SG * scores
        # per-subtile engine assignment
        e = np.zeros((KS, 256), np.float32)
        for t in range(NSUB):
            grp_i = t // GRP if t < NGRP * GRP else None
            rows = slice(t * 128, (t + 1) * 128)
            # note: node k = t*128+p lives at partition p, subtile t; scores
            # rows here are k-major which matches
            if grp_i is not None and grp_i in ACT_GROUPS:
                e[rows] = np.exp(s[rows] / SG).astype(NPF8).astype(np.float32)
            else:
                y = np.trunc(A8 * s[rows] + B8).astype(np.int8)
                e[rows] = y.view(NPF8).astype(np.float32)
        want[b] = e.T @ x32[b, :, :257]
    err = np.abs(got - want).max() / np.abs(want).max()
    print("CoreSim rel err vs bit-exact model:", err)
    assert err < 1e-3, err
    print("OK")


# revision 11
# speedup vs baseline: 2.0264x; 1.1821x over previous
"""Trainium2 Bass kernel for nn_Attention_9122510537215 (gnn_message_passing).

Math (per batch b):
    Q = query @ Wq.T + bq                  [LQ=256, 256]
    K = input @ Wk.T + bk                  [LK, 256]
    V = input @ Wv.T + bv                  [LK, 256]
    alpha = softmax_k(Q @ K.T / 16)        [256, LK]
    out[j] = sum_k alpha[j, k] * V[k, j]   [256]

Algebraic restructuring:
  * bk shifts every score column by a constant along k -> cancels in softmax_k.
  * G[b] = Wk.T @ (query_b @ Wq.T + bq).T / 16, so scoresT = input @ G  ([LK, 256]).
  * Instead of materializing V, accumulate H[j, i] = sum_k e[k, j] * input[k, i]
    (e = exp(scores)); numer[j] = sum_i H[j, i] * Wv[j, i]; an appended
    ones-column yields denom[j] = H[j, 256]; out = numer / denom + bv.
  * Softmax is computed unnormalized without max-subtraction (scores are O(1)).

Performance structure (vs the fp16 predecessor):
  * All matmuls run in fp8e4 (e4m3) with MatmulPerfMode.DoubleRow: two
    128-row contraction tiles per pass at 0.5 cycles/output-column.
      - scores: per 128-node subtile, ONE DR matmul contracts all 256 input
        features (xt laid out [i_lo(128 part), 2(i_hi), k]).
      - H: subtile PAIRS contract 256 nodes per pass (e laid out
        [k(128 part), 2(pair), j]; xn natural [k, 2(pair), 257]).
  * exp is the serial bottleneck (B*LQ*LK/8 = 6.4M exps/core), so it is
    SPLIT across two engines: ScalarE computes exact Exp (fp8 out,
    scale=1/SG), and VectorE computes a Schraudolph-style exp by writing
    round(A8*score + B8) as int8 and BITCASTING those bytes as fp8e4
    (weights' 3.3% rms wiggle averages out over 50k softmax terms).
  * Whole per-core input (12.9 MB fp8) is SBUF-resident; chunked DMAs on a
    single ordered sync queue overlap the whole compute pipeline.
  * Distribution: LK padded to 50176 = 8*6272, sharded over 8 cores; each
    core returns fp16 partial H [B, 2, 128, 257]; host reduces in float64.
"""

import numpy as np
from contextlib import ExitStack

import concourse.bass as bass
import concourse.mybir as mybir
import concourse.tile as tile
from concourse import bacc
from concourse.bass_utils import run_bass_kernel_spmd

# Problem constants (hardcoded; kernel.py must be self-contained).
B = 4
LQ = 256
LK = 50000
OUT = 256
KV = 256            # input feature dim
NORM = 1.0 / 16.0   # 1/sqrt(OUT)

N_CORES = 8
SUB = 128                  # nodes per subtile (PE contraction width)
NSUB = 49                  # subtiles per core per batch
KS = NSUB * SUB            # 6272 nodes per core per batch
LK_PAD = KS * N_CORES      # 50176
GRP = 4                    # subtiles per exp/psum group (2 DoubleRow pairs)
NGRP = NSUB // GRP         # 12 full groups; subtile 48 is the odd tail
CHUNKS = (12, 12, 12, 8, 5)  # subtiles per DMA chunk (small tail chunks
NCHUNK = len(CHUNKS)         # shrink the end-of-pipeline latency)
CH_OFF = tuple(sum(CHUNKS[:i]) for i in range(NCHUNK))

SG = 64.0                        # score scale inside PSUM (folded into g)
A8 = 8 * np.log2(np.e) / SG      # Schraudolph fp8e4 slope
B8 = 56.05                       # 8*7 bias, +0.5 trunc->round, -0.45 mean-cal

# 7 ScalarE groups / 5 VectorE groups (+ odd tail on VectorE) per batch,
# alternating so the two exp engines run concurrently.
ACT_GROUPS = frozenset((0, 2, 4, 6, 8, 10, 11))
PEND = 2                   # H matmuls lag their group by 2 so the PE FIFO
                           # never blocks the next group's scores on an exp

F8 = mybir.dt.float8e4
F16 = mybir.dt.float16
F32 = mybir.dt.float32
I8 = mybir.dt.int8
NPF8 = mybir.dt.np(mybir.dt.float8e4)
DR = mybir.MatmulPerfMode.DoubleRow


def _chunk_of(s):
    """DMA chunk index and subtile offset within the chunk."""
    for c in range(NCHUNK - 1, -1, -1):
        if s >= CH_OFF[c]:
            return c, s - CH_OFF[c]
    raise AssertionError(s)


def build():
    """Emit the per-core SPMD Bass module (identical on all cores)."""
    nc = bacc.Bacc("TRN2", target_bir_lowering=False, debug=False,
                   num_devices=N_CORES)
    xn = nc.dram_tensor("xn", [B, 128, NSUB, 258], F8, kind="ExternalInput")
    xt = nc.dram_tensor("xt", [B, 128, 2, KS], F8, kind="ExternalInput")
    # g[..., 0, :] = fp8(G), g[..., 1, :] = fp8(G - fp8(G)): a second
    # accumulating matmul restores ~11-bit G precision (the output error is
    # dominated by the NON-averaging diagonal term delta_G[q,q]*Var(x)).
    g = nc.dram_tensor("g", [128, B, 2, 2, 256], F8, kind="ExternalInput")
    ht = nc.dram_tensor("ht", [B, 2, 128, 257], F16, kind="ExternalOutput")

    with ExitStack() as ctx:
        tc = ctx.enter_context(tile.TileContext(nc))
        gp = ctx.enter_context(tc.tile_pool(name="gp", bufs=1))
        xnp = ctx.enter_context(tc.tile_pool(name="xnp", bufs=B * NCHUNK))
        xtp = ctx.enter_context(tc.tile_pool(name="xtp", bufs=B * NCHUNK))
        ep = ctx.enter_context(tc.tile_pool(name="ep", bufs=4))
        eip = ctx.enter_context(tc.tile_pool(name="eip", bufs=4))
        hop = ctx.enter_context(tc.tile_pool(name="hop", bufs=2))
        spp = ctx.enter_context(tc.tile_pool(name="spp", bufs=3, space="PSUM"))
        hpp = ctx.enter_context(tc.tile_pool(name="hpp", bufs=1, space="PSUM"))

        # G hi+lo for all batches, resident: [i_lo(128 part), b, i_hi, hl, q].
        g_sb = gp.tile([128, B, 2, 2, 256], F8)
        nc.sync.dma_start(out=g_sb[:, :, :, :, :], in_=g[:, :, :, :, :])

        # Chunked input loads, one ordered queue (sync/SP), interleaved so
        # batch 0's first chunks land first.
        xn_tiles = {}
        xt_tiles = {}
        cmax = max(CHUNKS)
        for b in range(B):
            for c in range(NCHUNK):
                s0, ns = CH_OFF[c], CHUNKS[c]
                xtt = xtp.tile([128, 2, cmax * SUB], F8, tag="xt")
                nc.sync.dma_start(
                    out=xtt[:, :, :ns * SUB],
                    in_=xt[b, :, :, s0 * SUB:(s0 + ns) * SUB])
                xt_tiles[(b, c)] = xtt
                xnt = xnp.tile([128, cmax, 258], F8, tag="xn")
                nc.sync.dma_start(out=xnt[:, :ns, :],
                                  in_=xn[b, :, s0:s0 + ns, :])
                xn_tiles[(b, c)] = xnt

        # groups: (subtile list, is_act) per batch; odd tail rides VectorE.
        groups = [([g_ * GRP + i for i in range(GRP)], g_ in ACT_GROUPS)
                  for g_ in range(NGRP)]
        groups.append(([NSUB - 1], False))

        for b in range(B):
            ht0 = hpp.tile([128, 257], F32, tag="ht0")
            ht1 = hpp.tile([128, 257], F32, tag="ht1")
            pend = []     # groups whose H matmuls haven't been issued yet
            popped = 0
            started = False

            for item in groups + [None] * PEND:
                if item is not None:
                    subs, is_act = item
                    sz = len(subs)
                    sp = spp.tile([128, GRP, 256], F32)
                    for i, s in enumerate(subs):
                        c, off = _chunk_of(s)
                        xtt = xt_tiles[(b, c)]
                        for hl in range(2):  # G hi then lo residual
                            nc.tensor.matmul(
                                sp[:, i, :],
                                xtt[:, :, off * SUB:(off + 1) * SUB],
                                g_sb[:, b, :, hl, :],
                                start=hl == 0, stop=hl == 1, perf_mode=DR)
                    if is_act:
                        e = ep.tile([128, GRP, 256], F8, tag="ea")
                        nc.scalar.activation(
                            e[:, :sz, :], sp[:, :sz, :],
                            mybir.ActivationFunctionType.Exp, scale=1.0 / SG)
                        e_f8 = e
                    else:
                        ei = eip.tile([128, GRP, 256], I8, tag="ei")
                        nc.vector.tensor_scalar(
                            ei[:, :sz, :], sp[:, :sz, :], A8, B8,
                            mybir.AluOpType.mult, mybir.AluOpType.add)
                        e_f8 = ei[:, :, :].bitcast(F8)
                    pend.append((e_f8, subs))
                # H matmuls lag PEND groups behind so the PE FIFO never
                # stalls the next group's scores on an exp result.
                if len(pend) > PEND or (item is None and pend):
                    pe_, psubs = pend.pop(0)
                    popped += 1
                    is_last_grp = popped == len(groups)
                    npair = len(psubs) // 2
                    for i in range(npair):
                        s = psubs[2 * i]
                        c, off = _chunk_of(s)
                        xnt = xn_tiles[(b, c)]
                        first = not started
                        started = True
                        last = (is_last_grp and i == npair - 1
                                and len(psubs) % 2 == 0)
                        for h, hacc in ((0, ht0), (1, ht1)):
                            nc.tensor.matmul(
                                hacc[:, :],
                                pe_[:, 2 * i:2 * i + 2, h * 128:(h + 1) * 128],
                                xnt[:, off:off + 2, 0:257],
                                start=first, stop=last, perf_mode=DR)
                    if len(psubs) % 2:  # odd tail: plain fp8 matmul
                        s = psubs[-1]
                        c, off = _chunk_of(s)
                        xnt = xn_tiles[(b, c)]
                        i = len(psubs) - 1
                        first = not started
                        started = True
                        for h, hacc in ((0, ht0), (1, ht1)):
                            nc.tensor.matmul(
                                hacc[:, :],
                                pe_[:, i, h * 128:(h + 1) * 128],
                                xnt[:, off, 0:257],
                                start=first, stop=is_last_grp)

            hts = hop.tile([128, 2, 257], F16)
            nc.vector.tensor_copy(hts[:, 0, :], ht0[:, :])
            nc.vector.tensor_copy(hts[:, 1, :], ht1[:, :])
            nc.gpsimd.dma_start(out=ht[b, 0], in_=hts[:, 0, :])
            nc.gpsimd.dma_start(out=ht[b, 1], in_=hts[:, 1, :])
    nc.compile()
    return nc


def _prepare_inputs(query, input, Wq, bq, Wk):
    """Host marshalling: folded G + fp8 input in both layouts, k-sharded."""
    # G[b] = Wk.T @ (query_b @ Wq.T + bq).T -> [B, 256(i), 256(q)], then
    # * NORM (1/16) * SG (64) so PSUM scores arrive pre-scaled by SG.
    Q = query.astype(np.float64) @ Wq.T.astype(np.float64) + bq
    G = np.einsum('di,bqd->biq', Wk.astype(np.float64), Q) * (NORM * SG)
    g_hi = G.astype(np.float32).astype(NPF8)
    g_lo = (G - g_hi.astype(np.float64)).astype(np.float32).astype(NPF8)
    # [i_lo, b, i_hi, hl, q] with i = i_hi*128 + i_lo
    g8 = np.ascontiguousarray(
        np.stack([g_hi, g_lo], axis=2)          # [B, 256, 2, 256]
        .reshape(B, 2, 128, 2, 256).transpose(2, 0, 1, 3, 4))

    xpad = np.zeros((B, LK_PAD, 258), np.float32)
    xpad[:, :LK, :256] = input
    xpad[:, :LK, 256] = 1.0   # ones-column -> denom; 0 on padded rows
    x8 = xpad.astype(NPF8)    # [B, LK_PAD, 258]

    in_maps = []
    for cid in range(N_CORES):
        sl = x8[:, cid * KS:(cid + 1) * KS, :]
        # natural: [B, 128(part), NSUB, 258]; node k = t*128 + p
        xn_c = sl.reshape(B, NSUB, 128, 258).transpose(0, 2, 1, 3)
        # transposed DoubleRow: [B, 128(i_lo), 2(i_hi), KS]
        xt_c = np.ascontiguousarray(
            sl[:, :, :256].transpose(0, 2, 1)).reshape(B, 2, 128, KS)
        xt_c = xt_c.transpose(0, 2, 1, 3)
        in_maps.append({
            "xn": np.ascontiguousarray(xn_c),
            "xt": np.ascontiguousarray(xt_c),
            "g": g8,
        })
    return in_maps


def kernel(query, input, Wq, bq, Wk, bk, Wv, bv):
    # bk provably cancels in softmax over k; bq folds into G; bv is applied
    # in the host epilogue.
    query = np.asarray(query, dtype=np.float32)
    input = np.asarray(input, dtype=np.float32)
    Wq = np.asarray(Wq, dtype=np.float32)
    bq = np.asarray(bq, dtype=np.float32)
    Wk = np.asarray(Wk, dtype=np.float32)
    Wv = np.asarray(Wv, dtype=np.float32)
    bv = np.asarray(bv, dtype=np.float32)

    nc = build()
    in_maps = _prepare_inputs(query, input, Wq, bq, Wk)
    res = run_bass_kernel_spmd(nc, in_maps, core_ids=list(range(N_CORES)))
    kernel._last_result = res

    numer = np.zeros((B, OUT))
    denom = np.zeros((B, OUT))
    Wv64 = Wv.astype(np.float64)
    for r in res.results:
        H = r["ht"].astype(np.float64).reshape(B, OUT, 257)  # j = h*128 + p
        numer += (H[:, :, :256] * Wv64[None]).sum(axis=2)
        denom += H[:, :, 256]
    out = numer / denom + bv
    return out.astype(np.float32)


if __name__ == "__main__":
    # CoreSim smoke test on the full module with random fp8 inputs.
    from concourse.bass_interp import CoreSim

    rng = np.random.default_rng(0)
    xpad = np.zeros((B, KS, 258), np.float32)
    xpad[:, :, :256] = rng.standard_normal((B, KS, 256))
    xpad[:, :, 256] = 1.0
    x8 = xpad.astype(NPF8)
    xn_np = np.ascontiguousarray(
        x8.reshape(B, NSUB, 128, 258).transpose(0, 2, 1, 3))
    xt_np = np.ascontiguousarray(
        x8[:, :, :256].transpose(0, 2, 1)).reshape(B, 2, 128, KS)
    xt_np = np.ascontiguousarray(xt_np.transpose(0, 2, 1, 3))
    g_np = (rng.standard_normal((B, 256, 256)) * 1.8).astype(np.float64)
    g_hi = g_np.astype(np.float32).astype(NPF8)
    g_lo = (g_np - g_hi.astype(np.float64)).astype(np.float32).astype(NPF8)
    g8 = np.ascontiguousarray(
        np.stack([g_hi, g_lo], axis=2)
        .reshape(B, 2, 128, 2, 256).transpose(2, 0, 1, 3, 4))

    nc = build()
    sim = CoreSim(nc)
    sim.tensor("xn")[:] = xn_np
    sim.tensor("xt")[:] = xt_np
    sim.tensor("g")[:] = g8
    sim.simulate()
    got = np.array(sim.tensor("ht")).astype(np.float64).reshape(B, OUT, 257)

    x32 = x8.astype(np.float32)
    gsum = (g8[:, :, :, 0, :].astype(np.float32)
            + g8[:, :, :, 1, :].astype(np.float32))  # [il, b, ih, q]
    want = np.zeros((B, OUT, 257))
    for b in range(B):
        gb = gsum[:, b, :, :].transpose(1, 0, 2).reshape(256, 256)
        s = x32[b, :, :256] @ gb  # = SG * scores
        # per-subtile engine assignment
        e = np.zeros((KS, 256), np.float32)
        for t in range(NSUB):
            grp_i = t // GRP if t < NGRP * GRP else None
            rows = slice(t * 128, (t + 1) * 128)
            # note: node k = t*128+p lives at partition p, subtile t; scores
            # rows here are k-major which matches
            if grp_i is not None and grp_i in ACT_GROUPS:
                e[rows] = np.exp(s[rows] / SG).astype(NPF8).astype(np.float32)
            else:
                y = np.trunc(A8 * s[rows] + B8).astype(np.int8)
                e[rows] = y.view(NPF8).astype(np.float32)
        want[b] = e.T @ x32[b, :, :257]
    err = np.abs(got - want).max() / np.abs(want).max()
    print("CoreSim rel err vs bit-exact model:", err)
    assert err < 1e-3, err
    print("OK")
